# revision 3
# baseline (speedup 1.0000x reference)
"""OctreeConv (gather + buffered GEMM) on 8 Trainium2 NeuronCores.

out[n, o] = sum_{k, c} data[neigh[n, k], c] * weights[k, c, o], with
neigh == -1 meaning "no neighbor" (contributes zero).

Strategy (data-parallel over nodes, SPMD on 8 cores):
  - Shard the 200000 nodes into 8 x 25000. Replicate data and weights.
  - -1 indices are remapped on the host to a zero row appended to data,
    so the gather needs no masking and no destination memset.
  - Per 512-node supertile on device:
      1 indirect DMA gathers 512*27 rows of 128B into [128, 4*864] SBUF
      (node mod 128 on partitions, (subtile, k, c) along free dim),
      PE transposes 128x128 blocks into [kc, node] PSUM tiles,
      DVE/ACT copy them to SBUF,
      7 accumulating matmuls with W_flat[864, 32] produce out^T [32, 512].
  - Host transposes/concats per-core outputs back to [200000, 32].
"""

import numpy as np

import concourse.bacc as bacc
import concourse.bass as bass
import concourse.mybir as mybir
from concourse.bass_utils import run_bass_kernel_spmd
from concourse.masks import make_identity
from concourse.tile import TileContext

N = 200000
K = 27
C = 32
NCORES = 8
NODES_PER_CORE = N // NCORES  # 25000
SUPER = 512                   # nodes per supertile
SUBT = SUPER // 128           # 4 sub-tiles of 128 nodes
NSUP = (NODES_PER_CORE + SUPER - 1) // SUPER  # 49
NODES_PAD = NSUP * SUPER      # 25088
KC = K * C                    # 864
NBLK = (KC + 127) // 128      # 7 contraction blocks
IDX_W = SUBT * K              # 108 indices per partition per supertile

_PROGRAM = None


def _build_program(reps: int = 1) -> bass.Bass:
    nc = bacc.Bacc("TRN2", target_bir_lowering=False, debug=False)
    f32 = mybir.dt.float32

    data = nc.dram_tensor("data", [N + 1, C], f32, kind="ExternalInput")
    wflat = nc.dram_tensor("wflat", [NBLK * 128, C], f32, kind="ExternalInput")
    nidx = nc.dram_tensor(
        "nidx", [128, NSUP * IDX_W], mybir.dt.int32, kind="ExternalInput"
    )
    out = nc.dram_tensor("out", [NSUP, C, SUPER], f32, kind="ExternalOutput")

    with TileContext(nc) as tc:
        with (
            tc.tile_pool(name="const", bufs=1) as cpool,
            tc.tile_pool(name="gpool", bufs=4) as gpool,
            tc.tile_pool(name="gtpool", bufs=4) as gtpool,
            tc.tile_pool(name="opool", bufs=3) as opool,
            tc.tile_pool(name="pst", bufs=2, space="PSUM") as pst,
            tc.tile_pool(name="pso", bufs=2, space="PSUM") as pso,
        ):
            ident = cpool.tile([128, 128], f32)
            make_identity(nc, ident)

            # w_sb[p, b, c] = wflat[b*128 + p, c]; one DMA for all blocks
            w_sb = cpool.tile([128, NBLK, C], f32)
            nc.sync.dma_start(
                out=w_sb[:],
                in_=wflat.rearrange("(b p) c -> p b c", p=128),
            )

            nidx_sb = cpool.tile([128, NSUP * IDX_W], mybir.dt.int32)
            nc.sync.dma_start(out=nidx_sb[:], in_=nidx[:])

            # PE warmup: observe the ident (Pool) and w_sb (DMA) semaphores
            # before the main loop. The hardware LDWEIGHTS slot only carries
            # a single sync wait, so each steady-state matmul may wait on at
            # most one semaphore.
            warm_ps = pst.tile([128, SUPER], f32)
            nc.tensor.transpose(
                out=warm_ps[:128, :128], in_=ident[:], identity=ident[:]
            )
            warm_po = pso.tile([C, SUPER], f32)
            nc.tensor.matmul(
                out=warm_po[:, :128],
                lhsT=w_sb[:, 0, :],
                rhs=ident[:],
                start=True,
                stop=True,
            )

            for T in [t for _ in range(reps) for t in range(NSUP)]:
                g = gpool.tile([128, SUBT * KC], f32)
                # One indirect DMA moves 128 rows (one index per partition):
                # gather slot (t, k) of this supertile per instruction.
                for t in range(SUBT):
                    for k in range(K):
                        col = T * IDX_W + t * K + k
                        nc.gpsimd.indirect_dma_start(
                            out=g[:, (t * K + k) * C : (t * K + k + 1) * C],
                            out_offset=None,
                            in_=data[:],
                            in_offset=bass.IndirectOffsetOnAxis(
                                ap=nidx_sb[:, col : col + 1],
                                axis=0,
                            ),
                        )

                opsum = pso.tile([C, SUPER], f32)
                for b in range(NBLK):
                    rows = min(128, KC - 128 * b)
                    gt_ps = pst.tile([128, SUPER], f32)
                    for t in range(SUBT):
                        nc.tensor.transpose(
                            out=gt_ps[:rows, t * 128 : (t + 1) * 128],
                            in_=g[:, t * KC + 128 * b : t * KC + 128 * b + rows],
                            identity=ident[:],
                        )
                    gt_sb = gtpool.tile([128, SUPER], f32)
                    if b % 2 == 0:
                        nc.vector.tensor_copy(out=gt_sb[:rows, :], in_=gt_ps[:rows, :])
                    else:
                        nc.scalar.copy(out=gt_sb[:rows, :], in_=gt_ps[:rows, :])
                    nc.tensor.matmul(
                        out=opsum[:],
                        lhsT=w_sb[:rows, b, :],
                        rhs=gt_sb[:rows, :],
                        start=(b == 0),
                        stop=(b == NBLK - 1),
                    )

                o_sb = opool.tile([C, SUPER], f32)
                nc.scalar.copy(out=o_sb[:], in_=opsum[:])
                nc.sync.dma_start(out=out[T], in_=o_sb[:])

    nc.compile()
    return nc


def _get_program() -> bass.Bass:
    global _PROGRAM
    if _PROGRAM is None:
        _PROGRAM = _build_program()
    return _PROGRAM


def _prep_core_inputs(data_pad, wflat, neigh32):
    """Build the 8 per-core input maps from full inputs."""
    in_maps = []
    for j in range(NCORES):
        shard = neigh32[j * NODES_PER_CORE : (j + 1) * NODES_PER_CORE]
        pad = np.full((NODES_PAD - NODES_PER_CORE, K), N, dtype=np.int32)
        shard = np.concatenate([shard, pad], axis=0)  # [25088, 27]
        # nidx[p, T*IDX_W + t*K + k] = shard[(T*SUBT + t)*128 + p, k]
        nidx = (
            shard.reshape(NSUP, SUBT, 128, K)
            .transpose(2, 0, 1, 3)
            .reshape(128, NSUP * IDX_W)
        )
        in_maps.append(
            {
                "data": data_pad,
                "wflat": wflat,
                "nidx": np.ascontiguousarray(nidx),
            }
        )
    return in_maps


def kernel(data, weights, neigh):
    data = np.asarray(data, dtype=np.float32)
    weights = np.asarray(weights, dtype=np.float32)
    neigh = np.asarray(neigh)

    data_pad = np.zeros((N + 1, C), dtype=np.float32)
    data_pad[:N] = data
    wflat = np.zeros((NBLK * 128, C), dtype=np.float32)
    wflat[:KC] = weights.reshape(KC, C)
    neigh32 = neigh.astype(np.int32)
    neigh32[neigh32 < 0] = N  # zero row

    nc = _get_program()
    in_maps = _prep_core_inputs(data_pad, wflat, neigh32)
    res = run_bass_kernel_spmd(nc, in_maps, core_ids=list(range(NCORES)))

    outs = []
    for j in range(NCORES):
        o = np.asarray(res.results[j]["out"])  # [NSUP, C, SUPER]
        o = o.transpose(0, 2, 1).reshape(NODES_PAD, C)[:NODES_PER_CORE]
        outs.append(o)
    return np.ascontiguousarray(np.concatenate(outs, axis=0), dtype=np.float32)

